# revision 9
# baseline (speedup 1.0000x reference)
"""AttnBlock (GroupNorm -> single-head attention over 64x64 pixels -> out conv
-> residual) on 8 Trainium2 NeuronCores.

Sharding: data parallel over batch (B=4) x 2-way split of the query-pixel axis
=> 8 cores, no collectives.  Each core receives its batch element's pixels as
two inputs: xq (its 2048 query columns) and xo (the other 2048 columns).  K/V
are computed over all 4096 pixels in the core-local order [xq | xo] (attention
sums over keys, so key ordering is irrelevant to the output).

All shapes hardcoded: B=4, C=512, H=W=64, N=4096, 32 groups.
"""

import numpy as np

B, C, H, W = 4, 512, 64, 64
N = H * W              # 4096 pixels
NQ = N // 2            # 2048 query pixels per core
NUM_GROUPS = 32
GSIZE = C // NUM_GROUPS  # 16 channels per group
EPS = 1e-6
SCALE = float(C) ** 0.5  # reference multiplies scores by sqrt(C)

P = 128                # partitions
CC = C // P            # 4 channel chunks
QCH = NQ // P          # 16 query chunks per core
NKQ = 1024             # k-columns per score quarter
NQW = N // NKQ         # 4 quarters per query chunk

# "fp32r" = fast reduced-precision fp32 matmul for the q/k/scores path;
# "fp32" = full precision (4x slower on PE for the scores matmuls).
QK_MODE = "fp32r"

_CACHE = {}


def _build(qk_mode):
    from contextlib import ExitStack

    import concourse.bacc as bacc
    import concourse.tile as tile
    from concourse import mybir
    from concourse.masks import make_identity

    dt = mybir.dt
    qk_dt = dt.float32r if qk_mode == "fp32r" else dt.float32

    nc = bacc.Bacc()
    xq_ext = nc.declare_dram_parameter("xq", [C, NQ], dt.float32, isOutput=False)
    xo_ext = nc.declare_dram_parameter("xo", [C, NQ], dt.float32, isOutput=False)
    wqT_ext = nc.declare_dram_parameter("wqT", [C, C], dt.float32, isOutput=False)
    wkT_ext = nc.declare_dram_parameter("wkT", [C, C], dt.float32, isOutput=False)
    wvT_ext = nc.declare_dram_parameter("wvT", [C, C], dt.float32, isOutput=False)
    woT_ext = nc.declare_dram_parameter("woT", [C, C], dt.float32, isOutput=False)
    biases_ext = nc.declare_dram_parameter("biases", [C, 4], dt.float32, isOutput=False)
    gn_ab_ext = nc.declare_dram_parameter("gn_ab", [C, 2], dt.float32, isOutput=False)
    gsel_ext = nc.declare_dram_parameter("gsel", [C, NUM_GROUPS], dt.float32, isOutput=False)
    esel_ext = nc.declare_dram_parameter("esel", [NUM_GROUPS, C], dt.float32, isOutput=False)
    out_ext = nc.declare_dram_parameter("out", [C, NQ], dt.float32, isOutput=True)

    with tile.TileContext(nc) as tc:
        # LEFT side: long-lived pools (whole kernel / attention phase).
        # RIGHT side: phase-scoped pools (GN scratch, conv weights, h).
        top = ExitStack()
        const = top.enter_context(tc.tile_pool(name="const", bufs=1, side="left"))
        biases_sb = const.tile([P, CC, 4], dt.float32)  # [:, :, 0..3] = bq, bk, bv, bo
        nc.sync.dma_start(out=biases_sb[:], in_=biases_ext.rearrange("(c p) k -> p c k", p=P))
        k_pool = top.enter_context(tc.tile_pool(name="k_pool", bufs=1, side="left"))
        vT_pool = top.enter_context(tc.tile_pool(name="vT_pool", bufs=1, side="left"))
        q_pool = top.enter_context(tc.tile_pool(name="q_pool", bufs=1, side="left"))

        # ---------------- Phase 1: GroupNorm ----------------
        hq_stack = ExitStack()
        hq_pool = hq_stack.enter_context(tc.tile_pool(name="hq_pool", bufs=1, side="right"))
        ho_stack = ExitStack()
        ho_pool = ho_stack.enter_context(tc.tile_pool(name="ho_pool", bufs=1, side="right"))
        gn_stack = ExitStack()
        stat_pool = gn_stack.enter_context(tc.tile_pool(name="stat_pool", bufs=1, side="right"))
        small = gn_stack.enter_context(tc.tile_pool(name="small", bufs=1, side="right"))
        ps_small = gn_stack.enter_context(
            tc.tile_pool(name="ps_small", bufs=1, space="PSUM", side="right"))

        gsel_sb = small.tile([P, CC, NUM_GROUPS], dt.float32)
        nc.sync.dma_start(out=gsel_sb[:], in_=gsel_ext.rearrange("(c p) g -> p c g", p=P))
        esel_sb = small.tile([NUM_GROUPS, C], dt.float32)
        nc.sync.dma_start(out=esel_sb[:], in_=esel_ext[:])
        gn_ab_sb = small.tile([P, CC, 2], dt.float32)
        nc.sync.dma_start(out=gn_ab_sb[:], in_=gn_ab_ext.rearrange("(c p) k -> p c k", p=P))
        eps_sb = small.tile([NUM_GROUPS, 1], dt.float32)
        nc.vector.memset(eps_sb[:], EPS)

        xq_t, xo_t, mv_t = [], [], []
        for cc in range(CC):
            xqt = hq_pool.tile([P, NQ], qk_dt, name=f"hq_{cc}", tag=f"hq_{cc}")
            nc.sync.dma_start(out=xqt[:], in_=xq_ext[cc * P:(cc + 1) * P, :].bitcast(qk_dt))
            xot = ho_pool.tile([P, NQ], qk_dt, name=f"ho_{cc}", tag=f"ho_{cc}")
            nc.sync.dma_start(out=xot[:], in_=xo_ext[cc * P:(cc + 1) * P, :].bitcast(qk_dt))
            xq_t.append(xqt)
            xo_t.append(xot)
            xqf = xqt[:].bitcast(dt.float32)
            xof = xot[:].bitcast(dt.float32)
            stats = stat_pool.tile([P, 8, 6], dt.float32, name=f"st_{cc}", tag="st", bufs=2)
            for j in range(4):
                nc.vector.bn_stats(out=stats[:, j, :], in_=xqf[:, j * 512:(j + 1) * 512])
            for j in range(4):
                nc.vector.bn_stats(out=stats[:, 4 + j, :], in_=xof[:, j * 512:(j + 1) * 512])
            mv = stat_pool.tile([P, 2], dt.float32, name=f"mv_{cc}", tag=f"mv_{cc}")
            nc.vector.bn_aggr(out=mv[:], in_=stats[:])
            # mv[:,1] := var + mean^2  (per-channel second moment)
            sq = stat_pool.tile([P, 1], dt.float32, name=f"sq_{cc}", tag="sq", bufs=2)
            nc.vector.tensor_mul(sq[:], mv[:, 0:1], mv[:, 0:1])
            nc.vector.tensor_add(mv[:, 1:2], mv[:, 1:2], sq[:])
            mv_t.append(mv)

        # group stats: [32, 2] = sum_c gsel[c,g] * mv[c,:]   (gsel = 1/16)
        gps = ps_small.tile([NUM_GROUPS, 2], dt.float32, tag="gps")
        for cc in range(CC):
            nc.tensor.matmul(gps[:], gsel_sb[:, cc, :], mv_t[cc][:],
                             start=(cc == 0), stop=(cc == CC - 1))
        g_sb = small.tile([NUM_GROUPS, 2], dt.float32)
        nc.scalar.copy(g_sb[:], gps[:])
        # var_g = E[x^2]_g - mean_g^2 ; rstd = 1/sqrt(var+eps)
        gm2 = small.tile([NUM_GROUPS, 1], dt.float32)
        nc.vector.tensor_mul(gm2[:], g_sb[:, 0:1], g_sb[:, 0:1])
        grp = small.tile([NUM_GROUPS, 2], dt.float32)  # col0 = mean, col1 = rstd
        nc.vector.tensor_copy(grp[:, 0:1], g_sb[:, 0:1])
        varg = small.tile([NUM_GROUPS, 1], dt.float32)
        nc.vector.tensor_sub(varg[:], g_sb[:, 1:2], gm2[:])
        stdg = small.tile([NUM_GROUPS, 1], dt.float32)
        nc.scalar.activation(stdg[:], varg[:], mybir.ActivationFunctionType.Sqrt,
                             bias=eps_sb[:], scale=1.0)
        nc.vector.reciprocal(grp[:, 1:2], stdg[:])

        # broadcast group (mean, rstd) to channels; fold GN affine:
        # a = gnw*rstd ; b = gnb - mean*a ; h = a*x + b  (in place over x)
        for cc in range(CC):
            pcs = ps_small.tile([P, 2], dt.float32, tag="pcs", bufs=2)
            nc.tensor.matmul(pcs[:], esel_sb[:, cc * P:(cc + 1) * P], grp[:],
                             start=True, stop=True)
            pc = small.tile([P, 2], dt.float32, name=f"pc_{cc}", tag="pc", bufs=2)
            nc.scalar.copy(pc[:], pcs[:])
            ab = small.tile([P, 2], dt.float32, name=f"ab_{cc}", tag="ab", bufs=2)
            nc.vector.tensor_mul(ab[:, 0:1], gn_ab_sb[:, cc, 0:1], pc[:, 1:2])
            t0 = small.tile([P, 1], dt.float32, name=f"t0_{cc}", tag="t0", bufs=2)
            nc.vector.tensor_mul(t0[:], pc[:, 0:1], ab[:, 0:1])
            nc.vector.tensor_sub(ab[:, 1:2], gn_ab_sb[:, cc, 1:2], t0[:])
            nc.vector.tensor_scalar(out=xq_t[cc][:], in0=xq_t[cc][:].bitcast(dt.float32),
                                    scalar1=ab[:, 0:1], scalar2=ab[:, 1:2],
                                    op0=mybir.AluOpType.mult, op1=mybir.AluOpType.add)
            nc.vector.tensor_scalar(out=xo_t[cc][:], in0=xo_t[cc][:].bitcast(dt.float32),
                                    scalar1=ab[:, 0:1], scalar2=ab[:, 1:2],
                                    op0=mybir.AluOpType.mult, op1=mybir.AluOpType.add)

        gn_stack.close()
        hq_t = [t[:] for t in xq_t]
        ho_t = [t[:] for t in xo_t]

        def h_cols(cc, col0, width):
            """h[cc][:, col0:col0+width] in the core-local order [hq | ho];
            callers never straddle the NQ boundary."""
            if col0 < NQ:
                return hq_t[cc][:, col0:col0 + width]
            return ho_t[cc][:, col0 - NQ:col0 - NQ + width]

        # ---------------- Phase 2: K / V / Q convs ----------------
        conv_ps_stack = ExitStack()
        ps_conv = conv_ps_stack.enter_context(
            tc.tile_pool(name="ps_conv", bufs=4, space="PSUM", side="right"))

        wk_stack = ExitStack()
        wk_pool = wk_stack.enter_context(tc.tile_pool(name="wk_pool", bufs=1, side="right"))
        wkT_sb = wk_pool.tile([P, CC, C], qk_dt)
        nc.sync.dma_start(out=wkT_sb[:],
                          in_=wkT_ext.rearrange("(c p) o -> p c o", p=P).bitcast(qk_dt))

        k_t = [k_pool.tile([P, N], qk_dt, name=f"k_{oc}", tag=f"k_{oc}") for oc in range(CC)]
        for oc in range(CC):
            for ncol in range(N // 512):
                ps = ps_conv.tile([P, 512], dt.float32, tag="conv", bufs=4)
                for cc in range(CC):
                    nc.tensor.matmul(ps[:], wkT_sb[:, cc, oc * P:(oc + 1) * P],
                                     h_cols(cc, ncol * 512, 512),
                                     start=(cc == 0), stop=(cc == CC - 1))
                nc.vector.tensor_scalar(
                    out=k_t[oc][:, ncol * 512:(ncol + 1) * 512], in0=ps[:],
                    scalar1=biases_sb[:, oc, 1:2], scalar2=None,
                    op0=mybir.AluOpType.add)
        wk_stack.close()

        wv_stack = ExitStack()
        wv_pool = wv_stack.enter_context(tc.tile_pool(name="wv_pool", bufs=1, side="right"))
        wvT_sb = wv_pool.tile([P, CC, C], qk_dt)
        nc.sync.dma_start(out=wvT_sb[:],
                          in_=wvT_ext.rearrange("(c p) o -> p c o", p=P).bitcast(qk_dt))

        # vT[pix, c_out] = h^T wvT  (+bv folded into attn output later)
        vT_sb = vT_pool.tile([P, N // P, C], dt.bfloat16)
        for pc in range(N // P):
            ps = ps_conv.tile([P, C], dt.float32, tag="conv", bufs=4)
            for cc in range(CC):
                nc.tensor.matmul(ps[:], h_cols(cc, pc * P, P), wvT_sb[:, cc, :],
                                 start=(cc == 0), stop=(cc == CC - 1))
            nc.scalar.copy(vT_sb[:, pc, :], ps[:])
        wv_stack.close()
        ho_stack.close()

        wq_stack = ExitStack()
        wq_pool = wq_stack.enter_context(tc.tile_pool(name="wq_pool", bufs=1, side="right"))
        wqT_sb = wq_pool.tile([P, CC, C], qk_dt)
        nc.sync.dma_start(out=wqT_sb[:],
                          in_=wqT_ext.rearrange("(c p) o -> p c o", p=P).bitcast(qk_dt))

        q_t = [q_pool.tile([P, NQ], qk_dt, name=f"q_{oc}", tag=f"q_{oc}") for oc in range(CC)]
        for oc in range(CC):
            for ncol in range(NQ // 512):
                ps = ps_conv.tile([P, 512], dt.float32, tag="conv", bufs=4)
                for cc in range(CC):
                    nc.tensor.matmul(ps[:], wqT_sb[:, cc, oc * P:(oc + 1) * P],
                                     hq_t[cc][:, ncol * 512:(ncol + 1) * 512],
                                     start=(cc == 0), stop=(cc == CC - 1))
                nc.vector.tensor_scalar(
                    out=q_t[oc][:, ncol * 512:(ncol + 1) * 512], in0=ps[:],
                    scalar1=biases_sb[:, oc, 0:1], scalar2=None,
                    op0=mybir.AluOpType.add)
        wq_stack.close()
        hq_stack.close()
        conv_ps_stack.close()

        # ---------------- Phase 3: attention ----------------
        at_stack = ExitStack()
        at = at_stack.enter_context(tc.tile_pool(name="at", bufs=2, side="left"))
        wT_pool = at_stack.enter_context(tc.tile_pool(name="wT_pool", bufs=1, side="left"))
        out_pool = at_stack.enter_context(tc.tile_pool(name="out_pool", bufs=2, side="left"))
        ps_sc = at_stack.enter_context(
            tc.tile_pool(name="ps_sc", bufs=2, space="PSUM", side="left"))
        ps_tp = at_stack.enter_context(
            tc.tile_pool(name="ps_tp", bufs=2, space="PSUM", side="left"))
        ps_at = at_stack.enter_context(
            tc.tile_pool(name="ps_at", bufs=2, space="PSUM", side="left"))

        ident = at.tile([P, P], dt.bfloat16, tag="ident", bufs=1)
        make_identity(nc, ident[:])
        woT_sb = at.tile([P, CC, C], dt.bfloat16, tag="woT", bufs=1)
        nc.gpsimd.dma_start(out=woT_sb[:], in_=woT_ext.rearrange("(c p) o -> p c o", p=P))

        for qg in range(QCH // 4):  # groups of 4 query chunks (512 queries)
            wT_sb = wT_pool.tile([P, N // P, 512], dt.bfloat16, tag="wT")
            for qi4 in range(4):
                qi = qg * 4 + qi4
                # --- scores + online softmax over 4 quarters of k ---
                e_q = at.tile([P, NQW, NKQ], dt.bfloat16, tag="e", bufs=2)
                mq = at.tile([P, NQW], dt.float32, tag="mq")
                sq = at.tile([P, NQW], dt.float32, tag="sq")
                bias_t = at.tile([P, NQW], dt.float32, tag="bias")
                for w in range(NQW):
                    ps = ps_sc.tile([P, NKQ], dt.float32, tag="sc", bufs=2)
                    for half in range(2):
                        col0 = w * NKQ + half * 512
                        for cc in range(CC):
                            nc.tensor.matmul(
                                ps[:, half * 512:(half + 1) * 512],
                                q_t[cc][:, qi * P:(qi + 1) * P],
                                k_t[cc][:, col0:col0 + 512],
                                start=(cc == 0), stop=(cc == CC - 1))
                    nc.vector.reduce_max(out=mq[:, w:w + 1], in_=ps[:],
                                         axis=mybir.AxisListType.X)
                    nc.vector.tensor_scalar_mul(bias_t[:, w:w + 1], mq[:, w:w + 1],
                                                -SCALE)
                    nc.scalar.activation(
                        out=e_q[:, w, :], in_=ps[:],
                        func=mybir.ActivationFunctionType.Exp,
                        bias=bias_t[:, w:w + 1], scale=SCALE,
                        accum_out=sq[:, w:w + 1])
                # combine quarters: m = max_w mq ; alpha_w = exp(SCALE*(mq-m))/s
                m_t = at.tile([P, 1], dt.float32, tag="m")
                nc.vector.reduce_max(out=m_t[:], in_=mq[:], axis=mybir.AxisListType.X)
                mb = at.tile([P, 1], dt.float32, tag="mb")
                nc.vector.tensor_scalar_mul(mb[:], m_t[:], -SCALE)
                beta = at.tile([P, NQW], dt.float32, tag="beta")
                nc.scalar.activation(out=beta[:], in_=mq[:],
                                     func=mybir.ActivationFunctionType.Exp,
                                     bias=mb[:], scale=SCALE)
                sb_t = at.tile([P, NQW], dt.float32, tag="sbt")
                nc.vector.tensor_mul(sb_t[:], sq[:], beta[:])
                s_t = at.tile([P, 1], dt.float32, tag="s")
                nc.vector.reduce_sum(out=s_t[:], in_=sb_t[:], axis=mybir.AxisListType.X)
                rs = at.tile([P, 1], dt.float32, tag="rs")
                nc.vector.reciprocal(rs[:], s_t[:])
                alpha = at.tile([P, NQW], dt.float32, tag="alpha")
                nc.vector.tensor_scalar_mul(alpha[:], beta[:], rs[:])
                # normalize e, then transpose into wT columns for this chunk
                for w in range(NQW):
                    nc.vector.tensor_scalar_mul(e_q[:, w, :], e_q[:, w, :],
                                                alpha[:, w:w + 1])
                for w in range(NQW):
                    for t2 in range(2):
                        tp = ps_tp.tile([P, 512], dt.bfloat16, tag="tp", bufs=2)
                        for j in range(4):
                            nc.tensor.transpose(
                                tp[:, j * P:(j + 1) * P],
                                e_q[:, w, (t2 * 4 + j) * P:(t2 * 4 + j + 1) * P],
                                ident[:])
                        kc0 = w * 8 + t2 * 4
                        nc.scalar.copy(
                            wT_sb[:, kc0:kc0 + 4, qi4 * P:(qi4 + 1) * P], tp[:])

            # --- attn = v @ weights^T for this 512-query group ---
            attn_sb = at.tile([P, CC, 512], dt.bfloat16, tag="attn")
            for oc in range(CC):
                ps = ps_at.tile([P, 512], dt.float32, tag="at", bufs=2)
                for kc in range(N // P):
                    nc.tensor.matmul(ps[:], vT_sb[:, kc, oc * P:(oc + 1) * P],
                                     wT_sb[:, kc, :],
                                     start=(kc == 0), stop=(kc == N // P - 1))
                # + bv (softmax weights sum to 1, so +bv[c] is exact)
                nc.vector.tensor_scalar(out=attn_sb[:, oc, :], in0=ps[:],
                                        scalar1=biases_sb[:, oc, 2:3], scalar2=None,
                                        op0=mybir.AluOpType.add)

            # --- out = wo @ attn + bo + xq ---
            for oc in range(CC):
                ps = ps_at.tile([P, 512], dt.float32, tag="at", bufs=2)
                for cc in range(CC):
                    nc.tensor.matmul(ps[:], woT_sb[:, cc, oc * P:(oc + 1) * P],
                                     attn_sb[:, cc, :],
                                     start=(cc == 0), stop=(cc == CC - 1))
                xq_sb = out_pool.tile([P, 512], dt.float32, tag="xq", bufs=2)
                nc.sync.dma_start(out=xq_sb[:],
                                  in_=xq_ext[oc * P:(oc + 1) * P, qg * 512:(qg + 1) * 512])
                o_sb = out_pool.tile([P, 512], dt.float32, tag="o", bufs=2)
                nc.vector.tensor_scalar(out=o_sb[:], in0=ps[:],
                                        scalar1=biases_sb[:, oc, 3:4], scalar2=None,
                                        op0=mybir.AluOpType.add)
                nc.vector.tensor_add(o_sb[:], o_sb[:], xq_sb[:])
                nc.sync.dma_start(
                    out=out_ext[oc * P:(oc + 1) * P, qg * 512:(qg + 1) * 512],
                    in_=o_sb[:])
        at_stack.close()
        top.close()

    nc.compile()
    return nc


def _get_nc(qk_mode):
    if qk_mode not in _CACHE:
        _CACHE[qk_mode] = _build(qk_mode)
    return _CACHE[qk_mode]


def kernel(x, gn_weight, gn_bias, wq, bq, wk, bk, wv, bv, wo, bo):
    from concourse.bass_utils import run_bass_kernel_spmd

    nc = _get_nc(QK_MODE)

    x = np.asarray(x, dtype=np.float32)
    f32 = lambda a: np.ascontiguousarray(np.asarray(a, dtype=np.float32))

    wqT = f32(np.asarray(wq, dtype=np.float32).T)
    wkT = f32(np.asarray(wk, dtype=np.float32).T)
    wvT = f32(np.asarray(wv, dtype=np.float32).T)
    woT = f32(np.asarray(wo, dtype=np.float32).T)
    biases = f32(np.stack([bq, bk, bv, bo], axis=1))        # [C, 4]
    gn_ab = f32(np.stack([gn_weight, gn_bias], axis=1))     # [C, 2]

    gsel = np.zeros((C, NUM_GROUPS), dtype=np.float32)
    gsel[np.arange(C), np.arange(C) // GSIZE] = 1.0 / GSIZE
    esel = np.zeros((NUM_GROUPS, C), dtype=np.float32)
    esel[np.arange(C) // GSIZE, np.arange(C)] = 1.0

    in_maps = []
    for core in range(8):
        b, half = core // 2, core % 2
        xb = x[b].reshape(C, N)
        xqb = f32(xb[:, half * NQ:(half + 1) * NQ])
        xob = f32(xb[:, (1 - half) * NQ:(2 - half) * NQ])
        in_maps.append({
            "xq": xqb, "xo": xob,
            "wqT": wqT, "wkT": wkT, "wvT": wvT, "woT": woT,
            "biases": biases, "gn_ab": gn_ab, "gsel": gsel, "esel": esel,
        })

    res = run_bass_kernel_spmd(nc, in_maps, core_ids=list(range(8)))

    out = np.empty((B, C, N), dtype=np.float32)
    for core in range(8):
        b, half = core // 2, core % 2
        out[b, :, half * NQ:(half + 1) * NQ] = res.results[core]["out"]
    return out.reshape(B, C, H, W)
